# revision 15
# baseline (speedup 1.0000x reference)
"""TRN2 Bass kernel for nn_BlockLBPUNet: BN(train) -> ternary 3x3 conv -> ReLU
-> residual add -> 1x1 conv + bias.

Sharding: data-parallel over batch (B=8 images -> 8 cores, one image per
core). BN batch statistics are computed per-core and combined with a tiny
[128,2] AllReduce. Weights are replicated.

Layout: channels (128) on partitions. Each core processes its image in
row-chunks; a chunk's normalized input xn lives in SBUF as a zero-padded
plane with row stride W+2=258 so that all 9 conv taps become constant
flat-offset shifts of a single plane. Matmuls run in float32r (full PE
rate, ~2e-4 max rel err vs fp32's 4x slower path).
"""
import sys

if '/opt/trn_rl_repo' not in sys.path:
    sys.path.insert(0, '/opt/trn_rl_repo')

import numpy as np
import concourse.bass as bass
import concourse.mybir as mybir
import concourse.tile as tile
from concourse import bacc, bass_utils

B, C, H, W = 8, 128, 256, 256
BN_EPS = 1e-5
R = 24          # output rows per chunk
GRP = 2         # psum tiles grouped per weight load
WP = W + 2      # padded row stride (258)
F32 = mybir.dt.float32
F32R = mybir.dt.float32r
ACT = mybir.ActivationFunctionType
ALU = mybir.AluOpType

_NC_CACHE = {}
NO_CC = False
IN_LOOP = False
PHASES = 'all'  # 'all' | 'stats' | 'conv'
TIMING_MODE = False  # True: x/out are Internal DRAM (no host I/O)
CONV_STAGE = 3  # 0=mm only, 1=+relu-res, 2=+1x1, 3=full


def _emit(nc, tc, x, gamma, beta, w3t, w1t, b1, out):
    LP = (R + 2) * WP + 2   # flat length of a padded x/xn chunk tile
    xf = x.ap().rearrange("c h w -> c (h w)")

    with (
        tc.tile_pool(name="singles", bufs=1) as singles,
        tc.tile_pool(name="dram", bufs=1, space="DRAM") as dram,
    ):
        # ---- replicated weights / consts ----
        wt_f = singles.tile([C, 9 * 128], F32)
        nc.sync.dma_start(out=wt_f, in_=w3t[:, :])
        w1_f = singles.tile([C, 128], F32)
        nc.sync.dma_start(out=w1_f, in_=w1t[:, :])
        wt = singles.tile([C, 9 * 128], F32R)
        nc.vector.tensor_copy(wt, wt_f)
        w1 = singles.tile([C, 128], F32R)
        nc.vector.tensor_copy(w1, w1_f)
        gm = singles.tile([C, 1], F32)
        nc.sync.dma_start(out=gm, in_=gamma[:, :])
        bt = singles.tile([C, 1], F32)
        nc.sync.dma_start(out=bt, in_=beta[:, :])
        bb = singles.tile([C, 1], F32)
        nc.sync.dma_start(out=bb, in_=b1[:, :])

        # ---- phase A: per-core BN stats over this core's image ----
        do_stats = PHASES in ('all', 'stats')
        do_conv = PHASES in ('all', 'conv')
        NPIX = H * W
        CH = 8192
        nch = NPIX // CH
        nsub = CH // 512
        stats = singles.tile([C, nch * nsub, 6], F32)
        mv = singles.tile([C, 2], F32)
        if do_stats:
            with tc.tile_pool(name="statsbuf", bufs=2) as spool:
                for ci in range(nch):
                    xt = spool.tile([C, CH], F32, tag="sx")
                    nc.sync.dma_start(out=xt, in_=xf[:, ci * CH:(ci + 1) * CH])
                    for j in range(nsub):
                        nc.vector.bn_stats(out=stats[:, ci * nsub + j, :],
                                           in_=xt[:, j * 512:(j + 1) * 512])
            nc.vector.bn_aggr(out=mv, in_=stats)
        else:
            nc.vector.memset(mv[:, 0:1], 0.0)
            nc.vector.memset(mv[:, 1:2], 1.0)

        # payload = [mean/B, (var + mean^2)/B]; allreduce-add -> [gmean, gEx2]
        pay = singles.tile([C, 2], F32)
        msq = singles.tile([C, 1], F32)
        nc.vector.tensor_mul(msq, mv[:, 0:1], mv[:, 0:1])
        nc.vector.tensor_scalar_mul(pay[:, 0:1], mv[:, 0:1], 1.0 / B)
        t2 = singles.tile([C, 1], F32)
        nc.vector.tensor_add(t2, mv[:, 1:2], msq)
        nc.vector.tensor_scalar_mul(pay[:, 1:2], t2, 1.0 / B)

        g = singles.tile([C, 2], F32)
        if NO_CC or IN_LOOP:
            nc.vector.tensor_scalar_mul(g, pay, float(B))
        else:
            cin = dram.tile([C, 2], F32)
            cout = dram.tile([C, 2], F32)
            nc.sync.dma_start(out=cin, in_=pay)
            nc.gpsimd.collective_compute(
                "AllReduce", ALU.add,
                replica_groups=[list(range(B))],
                ins=[cin.opt()], outs=[cout.opt()])
            nc.sync.dma_start(out=g, in_=cout)

        # a = gamma * rsqrt(var + eps); d = beta - a * gmean
        gm2 = singles.tile([C, 1], F32)
        nc.vector.tensor_mul(gm2, g[:, 0:1], g[:, 0:1])
        var = singles.tile([C, 1], F32)
        nc.vector.tensor_sub(var, g[:, 1:2], gm2)
        vare = singles.tile([C, 1], F32)
        nc.vector.tensor_scalar_add(vare, var, BN_EPS)
        sd = singles.tile([C, 1], F32)
        nc.scalar.sqrt(sd, vare)
        rstd = singles.tile([C, 1], F32)
        nc.vector.reciprocal(rstd, sd)
        a_t = singles.tile([C, 1], F32)
        nc.vector.tensor_mul(a_t, gm, rstd)
        am = singles.tile([C, 1], F32)
        nc.vector.tensor_mul(am, a_t, g[:, 0:1])
        d_t = singles.tile([C, 1], F32)
        nc.vector.tensor_sub(d_t, bt, am)

        # ---- phase C: conv pipeline over row chunks ----
        if not do_conv:
            return
        with (
            tc.tile_pool(name="xc", bufs=2) as xcp,
            tc.tile_pool(name="xnc", bufs=2) as xncp,
            tc.tile_pool(name="oc", bufs=2) as ocp,
            tc.tile_pool(name="yb", bufs=4) as ybp,
            tc.tile_pool(name="ps1", bufs=2 * GRP, space="PSUM") as ps1,
            tc.tile_pool(name="ps2", bufs=GRP + 2, space="PSUM") as ps2,
        ):
            r0 = 0
            while r0 < H:
                Rr = min(R, H - r0)
                x_t = xcp.tile([C, LP], F32, tag="xc")
                xn_t = xncp.tile([C, LP], F32R, tag="xnc")
                o_t = ocp.tile([C, R * WP], F32, tag="oc")

                xplane = x_t[:, 1:1 + (R + 2) * WP].rearrange(
                    "c (r q) -> c r q", q=WP)
                xnplane = xn_t[:, 1:1 + (R + 2) * WP].rearrange(
                    "c (r q) -> c r q", q=WP)

                # zero pad columns (and the 2 slack elements) of both planes
                for t, pl in ((x_t, xplane), (xn_t, xnplane)):
                    nc.gpsimd.memset(t[:, 0:1].bitcast(F32), 0.0)
                    nc.gpsimd.memset(t[:, LP - 1:LP].bitcast(F32), 0.0)
                    nc.gpsimd.memset(pl[:, :, 0:1].bitcast(F32), 0.0)
                    nc.gpsimd.memset(pl[:, :, 257:258].bitcast(F32), 0.0)

                # load x rows [r0-1, r0+Rr] clipped to the image
                lo_img = r0 - 1
                hi_img = r0 + Rr
                img_a = max(lo_img, 0)
                img_b = min(hi_img, H - 1)
                lo_row = img_a - lo_img          # chunk row of first DMA row
                nrows = img_b - img_a + 1
                nc.sync.dma_start(
                    out=xplane[:, lo_row:lo_row + nrows, 1:257],
                    in_=x[:, img_a:img_b + 1, :])

                # zero halo rows at image top/bottom
                if lo_img < 0:
                    nc.gpsimd.memset(xnplane[:, 0, :].bitcast(F32), 0.0)
                if hi_img > H - 1:
                    nc.gpsimd.memset(xnplane[:, Rr + 1, :].bitcast(F32), 0.0)

                # xn = a*x + d on the valid rows
                nc.scalar.activation(
                    out=xnplane[:, lo_row:lo_row + nrows, 1:257],
                    in_=xplane[:, lo_row:lo_row + nrows, 1:257],
                    func=ACT.Identity, bias=d_t[:, 0:1], scale=a_t[:, 0:1])

                # conv over 512-wide flat tiles of the padded out plane
                # split into <=512-wide tiles, avoiding any tile narrower
                # than 256 (float32r matmuls run 4x slower below 256)
                QT = Rr * WP
                widths = []
                rem = QT
                while rem > 0:
                    if rem > 512 and rem < 768:
                        widths.append((rem + 1) // 2)
                        widths.append(rem - (rem + 1) // 2)
                        rem = 0
                    else:
                        w_ = min(512, rem)
                        widths.append(w_)
                        rem -= w_
                # offsets of each tile
                offs = []
                q0 = 0
                for qn in widths:
                    offs.append((q0, qn))
                    q0 += qn

                # process tiles in pairs per tap so the stationary weight is
                # loaded once per 2 matmuls (halves LDWEIGHTS traffic)
                for gi in range(0, len(offs), GRP):
                    grp = offs[gi:gi + GRP]
                    p1s = [ps1.tile([C, 512], F32, tag="p1", name=f"p1_{gi}_{k}") for k in range(len(grp))]
                    for tap in range(9):
                        kh, kw = tap // 3, tap % 3
                        delta = kh * WP + kw
                        for (q0, qn), p1 in zip(grp, p1s):
                            rhs = xn_t[:, q0 + delta:q0 + delta + qn]
                            nc.tensor.matmul(
                                p1[:, :qn],
                                wt[:, tap * 128:(tap + 1) * 128],
                                rhs, start=(tap == 0), stop=(tap == 8))
                    if CONV_STAGE < 1:
                        continue
                    y_ts = []
                    for (q0, qn), p1 in zip(grp, p1s):
                        # y = relu(conv) + x
                        y_t = ybp.tile([C, 512], F32R, tag="y", name=f"y_{gi}_{q0}")
                        xs = x_t[:, q0 + WP + 1:q0 + WP + 1 + qn]
                        nc.vector.scalar_tensor_tensor(
                            out=y_t[:, :qn], in0=p1[:, :qn], scalar=0.0,
                            in1=xs, op0=ALU.max, op1=ALU.add)
                        y_ts.append(y_t)
                    if CONV_STAGE < 2:
                        continue
                    p2s = []
                    for (q0, qn), y_t in zip(grp, y_ts):
                        # 1x1 conv (stationary w1 across the group)
                        p2 = ps2.tile([C, 512], F32, tag="p2", name=f"p2_{gi}_{q0}")
                        nc.tensor.matmul(p2[:, :qn], w1, y_t[:, :qn],
                                         start=True, stop=True)
                        p2s.append(p2)
                    if CONV_STAGE < 3:
                        continue
                    for (q0, qn), p2 in zip(grp, p2s):
                        nc.scalar.activation(
                            out=o_t[:, q0:q0 + qn], in_=p2[:, :qn],
                            func=ACT.Identity, bias=bb[:, 0:1], scale=1.0)

                if CONV_STAGE >= 3:
                    oplane = o_t[:, 0:Rr * WP].rearrange(
                        "c (r q) -> c r q", q=WP)
                    nc.sync.dma_start(out=out[:, r0:r0 + Rr, :],
                                      in_=oplane[:, :, 1:257])
                r0 += Rr


def build(repeat=1, loop_n=0):
    key = (repeat, NO_CC, loop_n, GRP, R, PHASES, CONV_STAGE, TIMING_MODE)
    if key in _NC_CACHE:
        return _NC_CACHE[key]
    nc = bacc.Bacc("TRN2", target_bir_lowering=False, debug=False,
                   num_devices=B)
    if TIMING_MODE:
        dummy_in = nc.dram_tensor("dummy_in", [1, 1], F32,
                                  kind="ExternalInput")
        dummy_out = nc.dram_tensor("dummy_out", [1, 1], F32,
                                   kind="ExternalOutput")
        x = nc.dram_tensor("x", [C, H, W], F32)
        gamma = nc.dram_tensor("gamma", [C, 1], F32)
        beta = nc.dram_tensor("beta", [C, 1], F32)
        w3t = nc.dram_tensor("w3t", [C, 9 * 128], F32)
        w1t = nc.dram_tensor("w1t", [C, 128], F32)
        b1 = nc.dram_tensor("b1", [C, 1], F32)
        out = nc.dram_tensor("out", [C, H, W], F32)
    else:
        x = nc.dram_tensor("x", [C, H, W], F32, kind="ExternalInput")
        gamma = nc.dram_tensor("gamma", [C, 1], F32, kind="ExternalInput")
        beta = nc.dram_tensor("beta", [C, 1], F32, kind="ExternalInput")
        w3t = nc.dram_tensor("w3t", [C, 9 * 128], F32, kind="ExternalInput")
        w1t = nc.dram_tensor("w1t", [C, 128], F32, kind="ExternalInput")
        b1 = nc.dram_tensor("b1", [C, 1], F32, kind="ExternalInput")
        out = nc.dram_tensor("out", [C, H, W], F32, kind="ExternalOutput")

    with tile.TileContext(nc) as tc:
        if TIMING_MODE:
            with tc.tile_pool(name="dummy", bufs=1) as dp:
                dt_ = dp.tile([1, 1], F32)
                nc.sync.dma_start(out=dt_, in_=dummy_in[:, :])
                nc.sync.dma_start(out=dummy_out[:, :], in_=dt_)
        if loop_n:
            global IN_LOOP
            IN_LOOP = True
            try:
                with tc.For_i(0, loop_n, 1):
                    _emit(nc, tc, x, gamma, beta, w3t, w1t, b1, out)
            finally:
                IN_LOOP = False
        else:
            for _ in range(repeat):
                _emit(nc, tc, x, gamma, beta, w3t, w1t, b1, out)
    nc.compile()
    _NC_CACHE[key] = nc
    return nc


def make_in_maps(x, gamma, beta, w_lbp, w_1x1, b_1x1):
    x = np.ascontiguousarray(np.asarray(x, dtype=np.float32))
    w3t = np.ascontiguousarray(
        np.transpose(np.asarray(w_lbp, np.float32), (1, 2, 3, 0))
        .reshape(C, 9 * 128))
    w1t = np.ascontiguousarray(np.asarray(w_1x1, np.float32)[:, :, 0, 0].T)
    gamma = np.asarray(gamma, np.float32).reshape(C, 1)
    beta = np.asarray(beta, np.float32).reshape(C, 1)
    b1 = np.asarray(b_1x1, np.float32).reshape(C, 1)
    return [
        {"x": np.ascontiguousarray(x[b]), "gamma": gamma, "beta": beta,
         "w3t": w3t, "w1t": w1t, "b1": b1}
        for b in range(B)
    ]


def kernel(x, gamma, beta, w_lbp, w_1x1, b_1x1, repeat=1, loop_n=0):
    nc = build(repeat, loop_n)
    if TIMING_MODE:
        in_maps = [{"dummy_in": np.zeros((1, 1), np.float32)}
                   for _ in range(B)]
        res = bass_utils.run_bass_kernel_spmd(
            nc, in_maps, core_ids=list(range(B)), trace=False)
        return None
    in_maps = make_in_maps(x, gamma, beta, w_lbp, w_1x1, b_1x1)
    res = bass_utils.run_bass_kernel_spmd(
        nc, in_maps, core_ids=list(range(B)), trace=False)
    out = np.stack([res.results[b]["out"] for b in range(B)], axis=0)
    return out.astype(np.float32)
